# revision 16
# baseline (speedup 1.0000x reference)
"""LISTA (learned ISTA) sparse-coding forward pass on 8 Trainium2 NeuronCores.

Problem: I [4,1,192,192] -> im2col(9x9) -> 24 soft-thresholded iterations over
64 filters -> decode -> col2im overlap-add average -> [4,1,192,192].

Sharding: 8 cores = 4 images x 2 position-row halves (92 rows of 184 positions
each). Each core computes its full LISTA pipeline plus the col2im partial sums
for its 100-row output slab; the host merges the 8-row seams between the two
slabs of each image and divides by the overlap counts (pure unshard glue).

Algebra used (exact rewrites of the reference up to fp assoc.):
  - mean-subtraction folded into encoder:  c = WAc @ I_col,
      WAc = WA - rowmean(WA)  (since mean_patch = (1/81) * ones^T I_col)
  - iteration fused:  gamma_{t+1} = soft(S @ gamma_t + c),  S = I - WA@WD
  - gamma kept as a + bneg with a = relu(y-l) >= 0, bneg = min(y+l, 0) <= 0
    (soft(y) = a + bneg), so the subtraction never needs its own pass:
      y_{t+1} = Id@c + S@a_t + S@bneg_t   (3 accumulating PE passes)
  - decode: out_all = WW@a + WW@bneg + (J/81) @ I_col   (mean add-back)
"""

import contextlib
import numpy as np

# ---------------------------------------------------------------- constants
B, H, Wimg = 4, 192, 192
K = 9
F = 64
NCH = K * K  # 81
HO = H - K + 1  # 184
WO = Wimg - K + 1  # 184
UNF = 24
N_CORES = 8

ROWS = HO // 2  # 92 position rows per core
SLAB = ROWS + K - 1  # 100 image/output rows per core
NPOS = ROWS * WO  # 16928 positions per core
HALFR = ROWS // 2  # 46 rows per block-diag half
HALF = HALFR * WO  # 8464 columns per half

CH = 512
CHUNKS = [(i * CH, min((i + 1) * CH, HALF)) for i in range((HALF + CH - 1) // CH)]
SUPERS = [CHUNKS[i : i + 2] for i in range(0, len(CHUNKS), 2)]
DROWS = 2  # decode chunk = 2 position rows
DCH = DROWS * WO  # 368 columns

# weight blob layout: (name, partitions, cols) — bf16
BLOB_SPEC = [
    ("wac", NCH, F), ("sbd", 128, 128), ("id128", 128, 128),
    ("wwb", 128, 2 * NCH), ("eshb", ROWS, K * SLAB),
    ("bandf", SLAB, ROWS), ("bandb", ROWS, SLAB),
]
BLOBC = sum(nf for _, _, nf in BLOB_SPEC)

_STATE = {}


def _split_multi_waits(nc, mybir):
    """This walrus build supports a single sync-wait slot per instruction.
    Move extra waits onto preceding same-engine no-ops (same semantics:
    program order on one engine; all waits clear before the instruction)."""
    cnt = 0
    for fn in nc.m.functions:
        for bb in fn.blocks:
            insts = bb.instructions
            need = False
            for ins in insts:
                si = ins.sync_info
                if si is not None and si.on_wait is not None and len(si.on_wait) > 1:
                    need = True
                    break
            if not need:
                continue
            out = []
            for ins in insts:
                si = ins.sync_info
                if si is not None and si.on_wait is not None and len(si.on_wait) > 1:
                    waits = list(si.on_wait)
                    for w in waits[:-1]:
                        cnt += 1
                        nop = mybir.InstNoOp(name=f"wsplit-{cnt}", ins=[], outs=[])
                        nop.engine = ins.engine
                        nop.sync_info = mybir.SyncInfo(on_wait=[w], on_update=[])
                        out.append(nop)
                    ins.sync_info = mybir.SyncInfo(
                        on_wait=[waits[-1]], on_update=list(si.on_update or [])
                    )
                out.append(ins)
            bb.instructions = out
    return cnt


def _build(use_f32r=True):
    import concourse.bass as bass
    import concourse.mybir as mybir
    import concourse.tile as tile

    f32 = mybir.dt.float32
    bf16 = mybir.dt.bfloat16
    f32r = mybir.dt.float32r
    Alu = mybir.AluOpType
    Act = mybir.ActivationFunctionType

    nc = bass.Bass("TRN2", target_bir_lowering=False, debug=False)

    mmdt = bf16

    imgw = nc.dram_tensor("imgw", [K * SLAB * WO], mmdt, kind="ExternalInput").ap()
    blob_d = nc.dram_tensor("blob", [128, BLOBC], mmdt, kind="ExternalInput").ap()
    lams_d = nc.dram_tensor("lams", [128, 2], f32, kind="ExternalInput").ap()
    out_d = nc.dram_tensor("out", [SLAB, Wimg], f32, kind="ExternalOutput").ap()
    obuf = nc.dram_tensor("obuf", [NCH * NPOS], bf16, kind="Internal").ap()

    def r(ap):
        return ap

    with tile.TileContext(nc) as tc:
        with contextlib.ExitStack() as ctx:
            wpool = ctx.enter_context(tc.tile_pool(name="w", bufs=1))
            big = ctx.enter_context(tc.tile_pool(name="big", bufs=1))
            pp = ctx.enter_context(tc.tile_pool(name="ps", bufs=4, space="PSUM"))
            ring = ctx.enter_context(tc.tile_pool(name="ring", bufs=4))
            stg = ctx.enter_context(tc.tile_pool(name="stg", bufs=2))

            blob = wpool.tile([128, BLOBC], mmdt)
            nc.sync.dma_start(blob[:], blob_d)
            o = {}
            col = 0
            for name, np_, nf in BLOB_SPEC:
                o[name] = (np_, col, nf)
                col += nf
            def bl(name, cast=None):
                np_, c0, nf = o[name]
                v = blob[0:np_, c0:c0 + nf]
                return v.bitcast(f32) if cast else v
            wac = bl("wac"); sbd = bl("sbd")
            id128 = bl("id128"); wwb = bl("wwb")
            eshb = bl("eshb"); bandf = bl("bandf"); bandb = bl("bandb")
            lams = wpool.tile([128, 2], f32)
            nc.sync.dma_start(lams[:], lams_d)
            lam = lams[:, 0:1]
            nlam = lams[:, 1:2]

            icol = big.tile([NCH, NPOS], mmdt, tag="icol")
            slabt = wpool.tile([SLAB, 192], mmdt)
            rs_sb = wpool.tile([SLAB, HO], mmdt)
            mean_sb = wpool.tile([ROWS, HO], mmdt)
            rb_sb = wpool.tile([ROWS, 192], mmdt)
            c = big.tile([128, HALF], mmdt)
            gam = big.tile([128, HALF], mmdt)
            acc = big.tile([SLAB, Wimg], f32)
            ypool = ctx.enter_context(tc.tile_pool(name="y", bufs=4))

            # ---- im2col: host supplies img_w[kw] = slab[:, kw:kw+WO]; each
            # channel (kh, kw) = img_w[kw][kh:kh+ROWS] is one contiguous run.
            for hh in range(2):
                eng = nc.sync if hh == 0 else nc.scalar
                for q in range(2):
                    r0, r1 = hh * HALFR + q * (HALFR // 2), hh * HALFR + (q + 1) * (HALFR // 2)
                    eng.dma_start(
                        icol[:, r0 * WO:r1 * WO],
                        bass.AP(imgw.tensor, r0 * WO,
                                [[WO, K], [SLAB * WO, K], [1, (r1 - r0) * WO]]))
            nc.sync.dma_start(slabt[:, 0:WO],
                              bass.AP(imgw.tensor, 0, [[WO, SLAB], [1, WO]]))
            nc.sync.dma_start(slabt[:, WO:192],
                              bass.AP(imgw.tensor, (K - 1) * SLAB * WO + WO - 8,
                                      [[WO, SLAB], [1, 8]]))
            # ---- encode: c = WAc @ I_col, col-tiled concurrent pair
            # (h1 -> psum[0:64], h2 -> psum[64:128]); one bf16 copy, then
            # gamma0 = soft(c2) from the copy (Pool clip + DVE sub).
            for si, sup in enumerate(SUPERS):
                ps = pp.tile([128, 1024], f32, tag="ps")
                c0s, c1s = sup[0][0], sup[-1][1]
                for jj, (c0, c1) in enumerate(sup):
                    n = c1 - c0
                    nc.tensor.matmul(ps[0:F, jj * CH: jj * CH + n], r(wac),
                                     r(icol[:, c0:c1]), start=True, stop=True)
                    nc.tensor.matmul(ps[F:128, jj * CH: jj * CH + n], r(wac),
                                     r(icol[:, HALF + c0:HALF + c1]),
                                     start=True, stop=True)
                span = c1s - c0s
                nc.scalar.copy(c[:, c0s:c1s], ps[:, 0:span])
                z = ypool.tile([128, 1024], mmdt, tag="z0")
                nc.vector.tensor_scalar(z[:, 0:span], c[:, c0s:c1s],
                                        lam, nlam, Alu.min, Alu.max)
                nc.vector.tensor_tensor(gam[:, c0s:c1s], c[:, c0s:c1s],
                                        z[:, 0:span], Alu.subtract)

            # ---- mean path: bm = backward-box9(forward-box9(slab)/81);
            # replaces the decoder J-term; accumulated into col2im PSUM.
            ps1 = pp.tile([128, 1024], f32, tag="ps")
            id100 = id128[0:SLAB, 0:SLAB]
            for kw in range(K):
                nc.tensor.matmul(ps1[0:SLAB, 0:HO], id100, slabt[:, kw:kw + HO],
                                 start=(kw == 0), stop=(kw == K - 1))
            nc.vector.tensor_copy(rs_sb[:], ps1[0:SLAB, 0:HO])
            ps2 = pp.tile([128, 1024], f32, tag="ps")
            nc.tensor.matmul(ps2[0:ROWS, 0:HO], r(bandf), rs_sb[:],
                             start=True, stop=True)
            nc.vector.tensor_copy(mean_sb[:], ps2[0:ROWS, 0:HO])
            ps3 = pp.tile([128, 1024], f32, tag="ps")
            id92 = id128[0:ROWS, 0:ROWS]
            for kw in range(K):
                nc.tensor.matmul(ps3[0:ROWS, kw:kw + HO], id92, mean_sb[:],
                                 start=(kw == 0), stop=(kw == K - 1))
            nc.vector.tensor_copy(rb_sb[:], ps3[0:ROWS, 0:192])

            # ---- 23 fused iterations: y = Id@c + S@gam; gam' = y - clip(y)
            # ACT copies y out per 1024-superstep; DVE clip+sub batched in
            # 2048 pairs to amortize per-op overheads.
            for _t in range(UNF - 1):
                si = 0
                for sp in range(0, len(SUPERS), 2):
                    pair = SUPERS[sp:sp + 2]
                    p0 = pair[0][0][0]
                    yt = ypool.tile([128, 2048], mmdt, tag="y")
                    off = 0
                    for sup in pair:
                        ps = pp.tile([128, 1024], f32, tag="ps")
                        c0s, c1s = sup[0][0], sup[-1][1]
                        for jj, (c0, c1) in enumerate(sup):
                            nc.tensor.matmul(ps[:, jj * CH: jj * CH + (c1 - c0)],
                                             r(id128), r(c[:, c0:c1]),
                                             start=True, stop=False)
                        for jj, (c0, c1) in enumerate(sup):
                            nc.tensor.matmul(ps[:, jj * CH: jj * CH + (c1 - c0)],
                                             r(sbd), r(gam[:, c0:c1]),
                                             start=False, stop=True)
                        span = c1s - c0s
                        if span < CH:
                            nc.vector.tensor_copy(yt[:, off:off + span],
                                                  ps[:, 0:span])
                        else:
                            nc.scalar.copy(yt[:, off:off + span], ps[:, 0:span])
                        si += 1
                        off += span
                    z = ypool.tile([128, 2048], mmdt, tag="z")
                    nc.vector.tensor_scalar(z[:, 0:off], yt[:, 0:off],
                                            lam, nlam, Alu.min, Alu.max)
                    nc.vector.tensor_tensor(gam[:, p0:p0 + off], yt[:, 0:off],
                                            z[:, 0:off], Alu.subtract)

            # ---- decode: out_all = WW@gam only (mean term handled by the
            # bm path in col2im), stream to HBM row-major
            ngrp = HALFR // DROWS  # 23 two-row chunks per half
            di = 0
            for half in range(2):
                ww = wwb[:, 0:NCH] if half == 0 else wwb[:, NCH:2 * NCH]
                for g0 in range(0, ngrp, 2):
                    ps = pp.tile([128, 1024], f32, tag="ps")
                    rg = ring.tile([NCH, 2 * DCH], bf16, tag="ring")
                    nch = 0
                    for jj, g in enumerate(range(g0, min(g0 + 2, ngrp))):
                        c0 = g * DCH
                        sl = ps[0:NCH, jj * CH: jj * CH + DCH]
                        nc.tensor.matmul(sl, r(ww),
                                         r(gam[:, c0:c0 + DCH]),
                                         start=True, stop=True)
                        if di % 2 == 0:
                            nc.scalar.copy(rg[:, jj * DCH:(jj + 1) * DCH], sl)
                        else:
                            nc.vector.tensor_copy(rg[:, jj * DCH:(jj + 1) * DCH], sl)
                        di += 1
                        nch += 1
                    r0 = half * HALFR + g0 * DROWS
                    dst = bass.AP(obuf.tensor, r0 * NCH * WO,
                                  [[WO, NCH], [NCH * WO, nch * DROWS], [1, WO]])
                    eng = nc.sync if (g0 // 2) % 2 == 0 else nc.scalar
                    eng.dma_start(dst, rg[:, 0:nch * DCH])

            # ---- col2im: one contiguous gather (reuses icol's SBUF slot),
            # kw-merge per kh in the free dim, then row-shift via 0/1
            # shift-matrix matmuls accumulating in PSUM.
            stall = big.tile([ROWS, NCH * WO], bf16, tag="icol")
            for kh in range(K):
                eng = (nc.sync, nc.scalar, nc.gpsimd)[kh % 3]
                eng.dma_start(
                    stall[:, kh * K * WO:(kh + 1) * K * WO],
                    bass.AP(obuf.tensor, kh * K * WO,
                            [[NCH * WO, ROWS], [1, K * WO]]))
            ops = pp.tile([128, 1024], f32, tag="ps")
            for kh in range(K):
                lhs = eshb[:, kh * SLAB:(kh + 1) * SLAB]
                for kw in range(K):
                    nc.tensor.matmul(
                        ops[0:SLAB, kw:kw + WO], lhs,
                        stall[:, (kh * K + kw) * WO:(kh * K + kw + 1) * WO],
                        start=(kh == 0 and kw == 0), stop=False)
            nc.tensor.matmul(ops[0:SLAB, 0:192], r(bandb), rb_sb[:],
                             start=False, stop=True)
            nc.scalar.copy(acc[:], ops[0:SLAB, 0:Wimg])
            nc.sync.dma_start(out_d, acc[:])

    n = _split_multi_waits(nc, mybir)
    return nc


def _get_nc():
    if "nc" not in _STATE:
        _STATE["nc"] = _build(use_f32r=True)
    return _STATE["nc"]


def _make_in_maps(I, WA, WD, WW, lmbda):
    import ml_dtypes  # noqa: F401
    I = np.ascontiguousarray(np.asarray(I, np.float32))
    WA = np.asarray(WA, np.float32)
    WD = np.asarray(WD, np.float32)
    WW = np.asarray(WW, np.float32)
    lam = np.asarray(lmbda, np.float32).reshape(F)
    assert I.shape == (B, 1, H, Wimg)

    WAc = (WA - WA.mean(axis=1, keepdims=True)).astype(np.float32)  # [64,81]
    S = (np.eye(F, dtype=np.float32) - WA @ WD).astype(np.float32)  # [64,64]
    sbd = np.zeros((128, 128), np.float32)
    sbd[0:F, 0:F] = S.T
    sbd[F:128, F:128] = S.T
    id128 = np.eye(128, dtype=np.float32)
    wwb = np.zeros((128, 2 * NCH), np.float32)
    wwb[0:F, 0:NCH] = WW.T
    wwb[F:128, NCH:2 * NCH] = WW.T
    lam128 = np.concatenate([lam, lam]).reshape(128, 1).astype(np.float32)
    esh = np.zeros((ROWS, K * SLAB), np.float32)  # lhsT per kh: E[r, y]=1 iff y=r+kh
    for kh in range(K):
        for rr in range(ROWS):
            esh[rr, kh * SLAB + rr + kh] = 1.0
    bandf = np.zeros((SLAB, ROWS), np.float32)
    for rr in range(SLAB):
        for r in range(ROWS):
            if r <= rr <= r + K - 1:
                bandf[rr, r] = 1.0 / NCH
    bandb = np.zeros((ROWS, SLAB), np.float32)
    for r in range(ROWS):
        for y in range(SLAB):
            if 0 <= y - r <= K - 1:
                bandb[r, y] = 1.0
    vals = {"wac": WAc.T, "sbd": sbd, "id128": id128,
            "wwb": wwb, "eshb": esh, "bandf": bandf, "bandb": bandb}
    blob = np.zeros((128, BLOBC), np.float32)
    col = 0
    for name, np_, nf in BLOB_SPEC:
        v = np.asarray(vals[name], np.float32)
        assert v.shape == (np_, nf), (name, v.shape)
        blob[0:np_, col:col + nf] = v
        col += nf
    lams = np.concatenate([lam128, -lam128], axis=1).astype(np.float32)

    shared = {"blob": blob.astype(ml_dtypes.bfloat16), "lams": lams}
    in_maps = []
    for core in range(N_CORES):
        b, h = core // 2, core % 2
        r0 = h * ROWS
        slab = I[b, 0, r0:r0 + SLAB, :]
        imgw = np.stack([slab[:, kw:kw + WO] for kw in range(K)], axis=0)
        in_maps.append({"imgw": np.ascontiguousarray(imgw).reshape(-1).astype(
            ml_dtypes.bfloat16), **shared})
    return in_maps


def _unshard(results):
    cnt = np.zeros((H, Wimg), np.float32)
    for kh in range(K):
        for kw in range(K):
            cnt[kh:kh + HO, kw:kw + WO] += 1.0
    out = np.zeros((B, 1, H, Wimg), np.float32)
    for b in range(B):
        acc = np.zeros((H, Wimg), np.float32)
        acc[0:SLAB, :] += results[2 * b]["out"]
        acc[ROWS:ROWS + SLAB, :] += results[2 * b + 1]["out"]
        out[b, 0] = acc / cnt
    return out


def kernel(I, WA, WD, WW, lmbda, kernel_size=9, stride=1, unfoldings=24, **_kw):
    from concourse import bass_utils

    assert int(kernel_size) == K and int(stride) == 1 and int(unfoldings) == UNF
    in_maps = _make_in_maps(I, WA, WD, WW, lmbda)
    nc = _get_nc()
    last = None
    for _attempt in range(3):
        try:
            res = bass_utils.run_bass_kernel_spmd(
                nc, in_maps, core_ids=list(range(N_CORES)))
            return _unshard(res.results)
        except Exception as e:  # transient NRT device errors: retry
            last = e
    raise last



# revision 17
# speedup vs baseline: 1.0353x; 1.0353x over previous
"""LISTA (learned ISTA) sparse-coding forward pass on 8 Trainium2 NeuronCores.

Problem: I [4,1,192,192] -> im2col(9x9) -> 24 soft-thresholded iterations over
64 filters -> decode -> col2im overlap-add average -> [4,1,192,192].

Sharding: 8 cores = 4 images x 2 position-row halves (92 rows of 184 positions
each). Each core computes its full LISTA pipeline plus the col2im partial sums
for its 100-row output slab; the host merges the 8-row seams between the two
slabs of each image and divides by the overlap counts (pure unshard glue).

Algebra used (exact rewrites of the reference up to fp assoc.):
  - mean-subtraction folded into encoder:  c = WAc @ I_col,
      WAc = WA - rowmean(WA)  (since mean_patch = (1/81) * ones^T I_col)
  - iteration fused:  gamma_{t+1} = soft(S @ gamma_t + c),  S = I - WA@WD
  - gamma kept as a + bneg with a = relu(y-l) >= 0, bneg = min(y+l, 0) <= 0
    (soft(y) = a + bneg), so the subtraction never needs its own pass:
      y_{t+1} = Id@c + S@a_t + S@bneg_t   (3 accumulating PE passes)
  - decode: out_all = WW@a + WW@bneg + (J/81) @ I_col   (mean add-back)
"""

import contextlib
import numpy as np

# ---------------------------------------------------------------- constants
B, H, Wimg = 4, 192, 192
K = 9
F = 64
NCH = K * K  # 81
HO = H - K + 1  # 184
WO = Wimg - K + 1  # 184
UNF = 24
N_CORES = 8

ROWS = HO // 2  # 92 position rows per core
SLAB = ROWS + K - 1  # 100 image/output rows per core
NPOS = ROWS * WO  # 16928 positions per core
HALFR = ROWS // 2  # 46 rows per block-diag half
HALF = HALFR * WO  # 8464 columns per half

CH = 512
CHUNKS = [(i * CH, min((i + 1) * CH, HALF)) for i in range((HALF + CH - 1) // CH)]
SUPERS = [CHUNKS[i : i + 2] for i in range(0, len(CHUNKS), 2)]
DROWS = 2  # decode chunk = 2 position rows
DCH = DROWS * WO  # 368 columns

# weight blob layout: (name, partitions, cols) — bf16
BLOB_SPEC = [
    ("wac", NCH, F), ("sbd", 128, 128), ("id128", 128, 128),
    ("wwb", 128, 2 * NCH), ("eshb", ROWS, K * SLAB),
    ("bandb", ROWS, SLAB),
]
BLOBC = sum(nf for _, _, nf in BLOB_SPEC)

_STATE = {}


def _split_multi_waits(nc, mybir):
    """This walrus build supports a single sync-wait slot per instruction.
    Move extra waits onto preceding same-engine no-ops (same semantics:
    program order on one engine; all waits clear before the instruction)."""
    cnt = 0
    for fn in nc.m.functions:
        for bb in fn.blocks:
            insts = bb.instructions
            need = False
            for ins in insts:
                si = ins.sync_info
                if si is not None and si.on_wait is not None and len(si.on_wait) > 1:
                    need = True
                    break
            if not need:
                continue
            out = []
            for ins in insts:
                si = ins.sync_info
                if si is not None and si.on_wait is not None and len(si.on_wait) > 1:
                    waits = list(si.on_wait)
                    for w in waits[:-1]:
                        cnt += 1
                        nop = mybir.InstNoOp(name=f"wsplit-{cnt}", ins=[], outs=[])
                        nop.engine = ins.engine
                        nop.sync_info = mybir.SyncInfo(on_wait=[w], on_update=[])
                        out.append(nop)
                    ins.sync_info = mybir.SyncInfo(
                        on_wait=[waits[-1]], on_update=list(si.on_update or [])
                    )
                out.append(ins)
            bb.instructions = out
    return cnt


def _build(use_f32r=True):
    import concourse.bass as bass
    import concourse.mybir as mybir
    import concourse.tile as tile

    f32 = mybir.dt.float32
    bf16 = mybir.dt.bfloat16
    f32r = mybir.dt.float32r
    Alu = mybir.AluOpType
    Act = mybir.ActivationFunctionType

    nc = bass.Bass("TRN2", target_bir_lowering=False, debug=False)

    mmdt = bf16

    imgw = nc.dram_tensor("imgw", [K * SLAB * WO], mmdt, kind="ExternalInput").ap()
    blob_d = nc.dram_tensor("blob", [128, BLOBC], mmdt, kind="ExternalInput").ap()
    lams_d = nc.dram_tensor("lams", [128, 2], f32, kind="ExternalInput").ap()
    rbm_d = nc.dram_tensor("rbm", [ROWS, 192], bf16, kind="ExternalInput").ap()
    out_d = nc.dram_tensor("out", [SLAB, Wimg], f32, kind="ExternalOutput").ap()
    obuf = nc.dram_tensor("obuf", [NCH * NPOS], bf16, kind="Internal").ap()

    def r(ap):
        return ap

    with tile.TileContext(nc) as tc:
        with contextlib.ExitStack() as ctx:
            wpool = ctx.enter_context(tc.tile_pool(name="w", bufs=1))
            big = ctx.enter_context(tc.tile_pool(name="big", bufs=1))
            pp = ctx.enter_context(tc.tile_pool(name="ps", bufs=4, space="PSUM"))
            ring = ctx.enter_context(tc.tile_pool(name="ring", bufs=4))
            stg = ctx.enter_context(tc.tile_pool(name="stg", bufs=2))

            blob = wpool.tile([128, BLOBC], mmdt)
            nc.sync.dma_start(blob[:], blob_d)
            o = {}
            col = 0
            for name, np_, nf in BLOB_SPEC:
                o[name] = (np_, col, nf)
                col += nf
            def bl(name, cast=None):
                np_, c0, nf = o[name]
                v = blob[0:np_, c0:c0 + nf]
                return v.bitcast(f32) if cast else v
            wac = bl("wac"); sbd = bl("sbd")
            id128 = bl("id128"); wwb = bl("wwb")
            eshb = bl("eshb"); bandb = bl("bandb")
            lams = wpool.tile([128, 2], f32)
            nc.sync.dma_start(lams[:], lams_d)
            lam = lams[:, 0:1]
            nlam = lams[:, 1:2]

            icol = big.tile([NCH, NPOS], mmdt, tag="icol")
            rb_sb = wpool.tile([ROWS, 192], mmdt)
            c = big.tile([128, HALF], mmdt)
            gam = big.tile([128, HALF], mmdt)
            acc = big.tile([SLAB, Wimg], f32)
            ypool = ctx.enter_context(tc.tile_pool(name="y", bufs=4))

            # ---- im2col: host supplies img_w[kw] = slab[:, kw:kw+WO]; each
            # channel (kh, kw) = img_w[kw][kh:kh+ROWS] is one contiguous run.
            for hh in range(2):
                eng = nc.sync if hh == 0 else nc.scalar
                for q in range(2):
                    r0, r1 = hh * HALFR + q * (HALFR // 2), hh * HALFR + (q + 1) * (HALFR // 2)
                    eng.dma_start(
                        icol[:, r0 * WO:r1 * WO],
                        bass.AP(imgw.tensor, r0 * WO,
                                [[WO, K], [SLAB * WO, K], [1, (r1 - r0) * WO]]))
            nc.scalar.dma_start(rb_sb[:], rbm_d)
            # ---- encode: c = WAc @ I_col, col-tiled concurrent pair
            # (h1 -> psum[0:64], h2 -> psum[64:128]); one bf16 copy, then
            # gamma0 = soft(c2) from the copy (Pool clip + DVE sub).
            for si, sup in enumerate(SUPERS):
                ps = pp.tile([128, 1024], f32, tag="ps")
                c0s, c1s = sup[0][0], sup[-1][1]
                for jj, (c0, c1) in enumerate(sup):
                    n = c1 - c0
                    nc.tensor.matmul(ps[0:F, jj * CH: jj * CH + n], r(wac),
                                     r(icol[:, c0:c1]), start=True, stop=True)
                    nc.tensor.matmul(ps[F:128, jj * CH: jj * CH + n], r(wac),
                                     r(icol[:, HALF + c0:HALF + c1]),
                                     start=True, stop=True)
                span = c1s - c0s
                nc.scalar.copy(c[:, c0s:c1s], ps[:, 0:span])
                z = ypool.tile([128, 1024], mmdt, tag="z0")
                nc.vector.tensor_scalar(z[:, 0:span], c[:, c0s:c1s],
                                        lam, nlam, Alu.min, Alu.max)
                nc.vector.tensor_tensor(gam[:, c0s:c1s], c[:, c0s:c1s],
                                        z[:, 0:span], Alu.subtract)

            # ---- 23 fused iterations: y = Id@c + S@gam; gam' = y - clip(y)
            # ACT copies y out per 1024-superstep; DVE clip+sub batched in
            # 2048 pairs to amortize per-op overheads.
            for _t in range(UNF - 1):
                si = 0
                for sp in range(0, len(SUPERS), 2):
                    pair = SUPERS[sp:sp + 2]
                    p0 = pair[0][0][0]
                    yt = ypool.tile([128, 2048], mmdt, tag="y")
                    off = 0
                    for sup in pair:
                        ps = pp.tile([128, 1024], f32, tag="ps")
                        c0s, c1s = sup[0][0], sup[-1][1]
                        for jj, (c0, c1) in enumerate(sup):
                            nc.tensor.matmul(ps[:, jj * CH: jj * CH + (c1 - c0)],
                                             r(id128), r(c[:, c0:c1]),
                                             start=True, stop=False)
                        for jj, (c0, c1) in enumerate(sup):
                            nc.tensor.matmul(ps[:, jj * CH: jj * CH + (c1 - c0)],
                                             r(sbd), r(gam[:, c0:c1]),
                                             start=False, stop=True)
                        span = c1s - c0s
                        if span < CH:
                            nc.vector.tensor_copy(yt[:, off:off + span],
                                                  ps[:, 0:span])
                        else:
                            nc.scalar.copy(yt[:, off:off + span], ps[:, 0:span])
                        si += 1
                        off += span
                    z = ypool.tile([128, 2048], mmdt, tag="z")
                    nc.vector.tensor_scalar(z[:, 0:off], yt[:, 0:off],
                                            lam, nlam, Alu.min, Alu.max)
                    nc.vector.tensor_tensor(gam[:, p0:p0 + off], yt[:, 0:off],
                                            z[:, 0:off], Alu.subtract)

            # ---- decode: out_all = WW@gam only (mean term handled by the
            # bm path in col2im), stream to HBM row-major
            ngrp = HALFR // DROWS  # 23 two-row chunks per half
            di = 0
            for half in range(2):
                ww = wwb[:, 0:NCH] if half == 0 else wwb[:, NCH:2 * NCH]
                for g0 in range(0, ngrp, 2):
                    ps = pp.tile([128, 1024], f32, tag="ps")
                    rg = ring.tile([NCH, 2 * DCH], bf16, tag="ring")
                    nch = 0
                    for jj, g in enumerate(range(g0, min(g0 + 2, ngrp))):
                        c0 = g * DCH
                        sl = ps[0:NCH, jj * CH: jj * CH + DCH]
                        nc.tensor.matmul(sl, r(ww),
                                         r(gam[:, c0:c0 + DCH]),
                                         start=True, stop=True)
                        nch += 1
                    if di % 2 == 0:
                        nc.scalar.copy(
                            rg[:, 0:DCH], ps[0:NCH, 0:DCH])
                        if nch == 2:
                            nc.scalar.copy(
                                rg[:, DCH:2 * DCH], ps[0:NCH, CH:CH + DCH])
                    else:
                        nc.vector.tensor_copy(rg[:, 0:DCH], ps[0:NCH, 0:DCH])
                        if nch == 2:
                            nc.vector.tensor_copy(
                                rg[:, DCH:2 * DCH], ps[0:NCH, CH:CH + DCH])
                    di += 1
                    r0 = half * HALFR + g0 * DROWS
                    dst = bass.AP(obuf.tensor, r0 * NCH * WO,
                                  [[WO, NCH], [NCH * WO, nch * DROWS], [1, WO]])
                    eng = nc.sync if (g0 // 2) % 2 == 0 else nc.scalar
                    eng.dma_start(dst, rg[:, 0:nch * DCH])

            # ---- col2im: one contiguous gather (reuses icol's SBUF slot),
            # kw-merge per kh in the free dim, then row-shift via 0/1
            # shift-matrix matmuls accumulating in PSUM.
            stall = big.tile([ROWS, NCH * WO], bf16, tag="icol")
            for kh in range(K):
                eng = (nc.sync, nc.scalar, nc.gpsimd)[kh % 3]
                eng.dma_start(
                    stall[:, kh * K * WO:(kh + 1) * K * WO],
                    bass.AP(obuf.tensor, kh * K * WO,
                            [[NCH * WO, ROWS], [1, K * WO]]))
            ops = pp.tile([128, 1024], f32, tag="ps")
            for kh in range(K):
                lhs = eshb[:, kh * SLAB:(kh + 1) * SLAB]
                for kw in range(K):
                    nc.tensor.matmul(
                        ops[0:SLAB, kw:kw + WO], lhs,
                        stall[:, (kh * K + kw) * WO:(kh * K + kw + 1) * WO],
                        start=(kh == 0 and kw == 0), stop=False)
            nc.tensor.matmul(ops[0:SLAB, 0:192], r(bandb), rb_sb[:],
                             start=False, stop=True)
            nc.scalar.copy(acc[:], ops[0:SLAB, 0:Wimg])
            nc.sync.dma_start(out_d, acc[:])

    n = _split_multi_waits(nc, mybir)
    return nc


def _get_nc():
    if "nc" not in _STATE:
        _STATE["nc"] = _build(use_f32r=True)
    return _STATE["nc"]


def _make_in_maps(I, WA, WD, WW, lmbda):
    import ml_dtypes  # noqa: F401
    I = np.ascontiguousarray(np.asarray(I, np.float32))
    WA = np.asarray(WA, np.float32)
    WD = np.asarray(WD, np.float32)
    WW = np.asarray(WW, np.float32)
    lam = np.asarray(lmbda, np.float32).reshape(F)
    assert I.shape == (B, 1, H, Wimg)

    WAc = (WA - WA.mean(axis=1, keepdims=True)).astype(np.float32)  # [64,81]
    S = (np.eye(F, dtype=np.float32) - WA @ WD).astype(np.float32)  # [64,64]
    sbd = np.zeros((128, 128), np.float32)
    sbd[0:F, 0:F] = S.T
    sbd[F:128, F:128] = S.T
    id128 = np.eye(128, dtype=np.float32)
    wwb = np.zeros((128, 2 * NCH), np.float32)
    wwb[0:F, 0:NCH] = WW.T
    wwb[F:128, NCH:2 * NCH] = WW.T
    lam128 = np.concatenate([lam, lam]).reshape(128, 1).astype(np.float32)
    esh = np.zeros((ROWS, K * SLAB), np.float32)  # lhsT per kh: E[r, y]=1 iff y=r+kh
    for kh in range(K):
        for rr in range(ROWS):
            esh[rr, kh * SLAB + rr + kh] = 1.0
    bandb = np.zeros((ROWS, SLAB), np.float32)
    for r in range(ROWS):
        for y in range(SLAB):
            if 0 <= y - r <= K - 1:
                bandb[r, y] = 1.0
    vals = {"wac": WAc.T, "sbd": sbd, "id128": id128,
            "wwb": wwb, "eshb": esh, "bandb": bandb}
    blob = np.zeros((128, BLOBC), np.float32)
    col = 0
    for name, np_, nf in BLOB_SPEC:
        v = np.asarray(vals[name], np.float32)
        assert v.shape == (np_, nf), (name, v.shape)
        blob[0:np_, col:col + nf] = v
        col += nf
    lams = np.concatenate([lam128, -lam128], axis=1).astype(np.float32)

    shared = {"blob": blob.astype(ml_dtypes.bfloat16), "lams": lams}
    in_maps = []
    for core in range(N_CORES):
        b, h = core // 2, core % 2
        r0 = h * ROWS
        slab = I[b, 0, r0:r0 + SLAB, :].astype(ml_dtypes.bfloat16).astype(
            np.float32)
        imgw = np.stack([slab[:, kw:kw + WO] for kw in range(K)], axis=0)
        # host-side mean prefix: rbm[r, x] = sum_kw mean[r, x-kw] with
        # mean = forward-box9(slab)/81 (the device adds the column-band sum
        # in the col2im PSUM via the bandb matmul)
        rs = np.zeros((SLAB, HO), np.float32)
        for kw in range(K):
            rs += slab[:, kw:kw + HO]
        mean = np.zeros((ROWS, HO), np.float32)
        for kh in range(K):
            mean += rs[kh:kh + ROWS, :]
        mean = (mean / NCH).astype(ml_dtypes.bfloat16).astype(np.float32)
        rbm = np.zeros((ROWS, 192), np.float32)
        for kw in range(K):
            rbm[:, kw:kw + HO] += mean
        in_maps.append({"imgw": np.ascontiguousarray(imgw).reshape(-1).astype(
            ml_dtypes.bfloat16),
            "rbm": rbm.astype(ml_dtypes.bfloat16), **shared})
    return in_maps


def _unshard(results):
    cnt = np.zeros((H, Wimg), np.float32)
    for kh in range(K):
        for kw in range(K):
            cnt[kh:kh + HO, kw:kw + WO] += 1.0
    out = np.zeros((B, 1, H, Wimg), np.float32)
    for b in range(B):
        acc = np.zeros((H, Wimg), np.float32)
        acc[0:SLAB, :] += results[2 * b]["out"]
        acc[ROWS:ROWS + SLAB, :] += results[2 * b + 1]["out"]
        out[b, 0] = acc / cnt
    return out


def kernel(I, WA, WD, WW, lmbda, kernel_size=9, stride=1, unfoldings=24, **_kw):
    from concourse import bass_utils

    assert int(kernel_size) == K and int(stride) == 1 and int(unfoldings) == UNF
    in_maps = _make_in_maps(I, WA, WD, WW, lmbda)
    nc = _get_nc()
    last = None
    for _attempt in range(3):
        try:
            res = bass_utils.run_bass_kernel_spmd(
                nc, in_maps, core_ids=list(range(N_CORES)))
            return _unshard(res.results)
        except Exception as e:  # transient NRT device errors: retry
            last = e
    raise last



# revision 19
# speedup vs baseline: 1.0364x; 1.0011x over previous
"""LISTA (learned ISTA) sparse-coding forward pass on 8 Trainium2 NeuronCores.

Problem: I [4,1,192,192] -> im2col(9x9) -> 24 soft-thresholded iterations over
64 filters -> decode -> col2im overlap-add average -> [4,1,192,192].

Sharding: 8 cores = 4 images x 2 position-row halves (92 rows of 184 positions
each). Each core computes its full LISTA pipeline plus the col2im partial sums
for its 100-row output slab; the host merges the 8-row seams between the two
slabs of each image and divides by the overlap counts (pure unshard glue).

Algebra used (exact rewrites of the reference up to fp assoc.):
  - mean-subtraction folded into encoder:  c = WAc @ I_col,
      WAc = WA - rowmean(WA)  (since mean_patch = (1/81) * ones^T I_col)
  - iteration fused:  gamma_{t+1} = soft(S @ gamma_t + c),  S = I - WA@WD
  - gamma kept as a + bneg with a = relu(y-l) >= 0, bneg = min(y+l, 0) <= 0
    (soft(y) = a + bneg), so the subtraction never needs its own pass:
      y_{t+1} = Id@c + S@a_t + S@bneg_t   (3 accumulating PE passes)
  - decode: out_all = WW@a + WW@bneg + (J/81) @ I_col   (mean add-back)
"""

import contextlib
import numpy as np

# ---------------------------------------------------------------- constants
B, H, Wimg = 4, 192, 192
K = 9
F = 64
NCH = K * K  # 81
HO = H - K + 1  # 184
WO = Wimg - K + 1  # 184
UNF = 24
N_CORES = 8

ROWS = HO // 2  # 92 position rows per core
SLAB = ROWS + K - 1  # 100 image/output rows per core
NPOS = ROWS * WO  # 16928 positions per core
HALFR = ROWS // 2  # 46 rows per block-diag half
HALF = HALFR * WO  # 8464 columns per half

CH = 512
CHUNKS = [(i * CH, min((i + 1) * CH, HALF)) for i in range((HALF + CH - 1) // CH)]
SUPERS = [CHUNKS[i : i + 2] for i in range(0, len(CHUNKS), 2)]
DROWS = 2  # decode chunk = 2 position rows
DCH = DROWS * WO  # 368 columns

# weight blob layout: (name, partitions, cols) — bf16
BLOB_SPEC = [
    ("wac", NCH, F), ("sbd", 128, 128), ("id128", 128, 128),
    ("wwb", 128, 2 * NCH), ("eshb", ROWS, K * SLAB),
    ("bandb", ROWS, SLAB),
]
BLOBC = sum(nf for _, _, nf in BLOB_SPEC)

_STATE = {}


def _split_multi_waits(nc, mybir):
    """This walrus build supports a single sync-wait slot per instruction.
    Move extra waits onto preceding same-engine no-ops (same semantics:
    program order on one engine; all waits clear before the instruction)."""
    cnt = 0
    for fn in nc.m.functions:
        for bb in fn.blocks:
            insts = bb.instructions
            need = False
            for ins in insts:
                si = ins.sync_info
                if si is not None and si.on_wait is not None and len(si.on_wait) > 1:
                    need = True
                    break
            if not need:
                continue
            out = []
            for ins in insts:
                si = ins.sync_info
                if si is not None and si.on_wait is not None and len(si.on_wait) > 1:
                    waits = list(si.on_wait)
                    for w in waits[:-1]:
                        cnt += 1
                        nop = mybir.InstNoOp(name=f"wsplit-{cnt}", ins=[], outs=[])
                        nop.engine = ins.engine
                        nop.sync_info = mybir.SyncInfo(on_wait=[w], on_update=[])
                        out.append(nop)
                    ins.sync_info = mybir.SyncInfo(
                        on_wait=[waits[-1]], on_update=list(si.on_update or [])
                    )
                out.append(ins)
            bb.instructions = out
    return cnt


def _build(use_f32r=True):
    import concourse.bass as bass
    import concourse.mybir as mybir
    import concourse.tile as tile

    f32 = mybir.dt.float32
    bf16 = mybir.dt.bfloat16
    f32r = mybir.dt.float32r
    Alu = mybir.AluOpType
    Act = mybir.ActivationFunctionType

    nc = bass.Bass("TRN2", target_bir_lowering=False, debug=False)

    mmdt = bf16

    imgw = nc.dram_tensor("imgw", [K * SLAB * WO], mmdt, kind="ExternalInput").ap()
    blob_d = nc.dram_tensor("blob", [128, BLOBC], mmdt, kind="ExternalInput").ap()
    lams_d = nc.dram_tensor("lams", [128, 2], f32, kind="ExternalInput").ap()
    rbm_d = nc.dram_tensor("rbm", [ROWS, 192], bf16, kind="ExternalInput").ap()
    out_d = nc.dram_tensor("out", [SLAB, Wimg], f32, kind="ExternalOutput").ap()
    obuf = nc.dram_tensor("obuf", [NCH * NPOS], bf16, kind="Internal").ap()

    def r(ap):
        return ap

    with tile.TileContext(nc) as tc:
        with contextlib.ExitStack() as ctx:
            wpool = ctx.enter_context(tc.tile_pool(name="w", bufs=1))
            big = ctx.enter_context(tc.tile_pool(name="big", bufs=1))
            pp = ctx.enter_context(tc.tile_pool(name="ps", bufs=4, space="PSUM"))
            ring = ctx.enter_context(tc.tile_pool(name="ring", bufs=4))
            stg = ctx.enter_context(tc.tile_pool(name="stg", bufs=2))

            blob = wpool.tile([128, BLOBC], mmdt)
            nc.sync.dma_start(blob[:], blob_d)
            o = {}
            col = 0
            for name, np_, nf in BLOB_SPEC:
                o[name] = (np_, col, nf)
                col += nf
            def bl(name, cast=None):
                np_, c0, nf = o[name]
                v = blob[0:np_, c0:c0 + nf]
                return v.bitcast(f32) if cast else v
            wac = bl("wac"); sbd = bl("sbd")
            id128 = bl("id128"); wwb = bl("wwb")
            eshb = bl("eshb"); bandb = bl("bandb")
            lams = wpool.tile([128, 2], f32)
            nc.sync.dma_start(lams[:], lams_d)
            lam = lams[:, 0:1]
            nlam = lams[:, 1:2]

            icol = big.tile([NCH, NPOS], mmdt, tag="icol")
            rb_sb = wpool.tile([ROWS, 192], mmdt)
            c = big.tile([128, HALF], mmdt)
            gam = big.tile([128, HALF], mmdt)
            acc = big.tile([SLAB, Wimg], f32)
            ypool = ctx.enter_context(tc.tile_pool(name="y", bufs=4))

            # ---- im2col: host supplies img_w[kw] = slab[:, kw:kw+WO]; each
            # channel (kh, kw) = img_w[kw][kh:kh+ROWS] is one contiguous run.
            qeng = [nc.sync, nc.gpsimd, nc.scalar, nc.sync]
            for hh in range(2):
                for q in range(2):
                    r0, r1 = hh * HALFR + q * (HALFR // 2), hh * HALFR + (q + 1) * (HALFR // 2)
                    qeng[hh * 2 + q].dma_start(
                        icol[:, r0 * WO:r1 * WO],
                        bass.AP(imgw.tensor, r0 * WO,
                                [[WO, K], [SLAB * WO, K], [1, (r1 - r0) * WO]]))
            nc.scalar.dma_start(rb_sb[:], rbm_d)
            # ---- encode: c = WAc @ I_col, col-tiled concurrent pair
            # (h1 -> psum[0:64], h2 -> psum[64:128]); one bf16 copy, then
            # gamma0 = soft(c2) from the copy (Pool clip + DVE sub).
            for si, sup in enumerate(SUPERS):
                ps = pp.tile([128, 1024], f32, tag="ps")
                c0s, c1s = sup[0][0], sup[-1][1]
                for jj, (c0, c1) in enumerate(sup):
                    n = c1 - c0
                    nc.tensor.matmul(ps[0:F, jj * CH: jj * CH + n], r(wac),
                                     r(icol[:, c0:c1]), start=True, stop=True)
                    nc.tensor.matmul(ps[F:128, jj * CH: jj * CH + n], r(wac),
                                     r(icol[:, HALF + c0:HALF + c1]),
                                     start=True, stop=True)
                span = c1s - c0s
                nc.scalar.copy(c[:, c0s:c1s], ps[:, 0:span])
                z = ypool.tile([128, 1024], mmdt, tag="z0")
                nc.vector.tensor_scalar(z[:, 0:span], c[:, c0s:c1s],
                                        lam, nlam, Alu.min, Alu.max)
                nc.vector.tensor_tensor(gam[:, c0s:c1s], c[:, c0s:c1s],
                                        z[:, 0:span], Alu.subtract)

            # ---- 23 fused iterations: y = Id@c + S@gam; gam' = y - clip(y)
            # ACT copies y out per 1024-superstep; DVE clip+sub batched in
            # 2048 pairs to amortize per-op overheads.
            for _t in range(UNF - 1):
                si = 0
                for sp in range(0, len(SUPERS), 2):
                    pair = SUPERS[sp:sp + 2]
                    p0 = pair[0][0][0]
                    yt = ypool.tile([128, 2048], mmdt, tag="y")
                    off = 0
                    tails = []
                    for sup in pair:
                        ps = pp.tile([128, 1024], f32, tag="ps")
                        c0s, c1s = sup[0][0], sup[-1][1]
                        for jj, (c0, c1) in enumerate(sup):
                            nc.tensor.matmul(ps[:, jj * CH: jj * CH + (c1 - c0)],
                                             r(id128), r(c[:, c0:c1]),
                                             start=True, stop=False)
                        for jj, (c0, c1) in enumerate(sup):
                            nc.tensor.matmul(ps[:, jj * CH: jj * CH + (c1 - c0)],
                                             r(sbd), r(gam[:, c0:c1]),
                                             start=False, stop=True)
                        span = c1s - c0s
                        if span < CH:
                            # tail superstep: DVE clip+sub straight from PSUM
                            tails.append((ps, c0s, span))
                        else:
                            nc.scalar.copy(yt[:, off:off + span], ps[:, 0:span])
                            off += span
                    if off:
                        z = ypool.tile([128, 2048], mmdt, tag="z")
                        nc.vector.tensor_scalar(z[:, 0:off], yt[:, 0:off],
                                                lam, nlam, Alu.min, Alu.max)
                        nc.vector.tensor_tensor(gam[:, p0:p0 + off],
                                                yt[:, 0:off],
                                                z[:, 0:off], Alu.subtract)
                    for (ps, c0s, span) in tails:
                        zt = ypool.tile([128, 2048], mmdt, tag="z")
                        nc.vector.tensor_scalar(zt[:, 0:span], ps[:, 0:span],
                                                lam, nlam, Alu.min, Alu.max)
                        nc.vector.tensor_tensor(gam[:, c0s:c0s + span],
                                                ps[:, 0:span],
                                                zt[:, 0:span], Alu.subtract)

            # ---- decode: out_all = WW@gam only (mean term handled by the
            # bm path in col2im), stream to HBM row-major
            ngrp = HALFR // DROWS  # 23 two-row chunks per half
            di = 0
            for half in range(2):
                ww = wwb[:, 0:NCH] if half == 0 else wwb[:, NCH:2 * NCH]
                for g0 in range(0, ngrp, 2):
                    ps = pp.tile([128, 1024], f32, tag="ps")
                    rg = ring.tile([NCH, 2 * DCH], bf16, tag="ring")
                    nch = 0
                    for jj, g in enumerate(range(g0, min(g0 + 2, ngrp))):
                        c0 = g * DCH
                        sl = ps[0:NCH, jj * CH: jj * CH + DCH]
                        nc.tensor.matmul(sl, r(ww),
                                         r(gam[:, c0:c0 + DCH]),
                                         start=True, stop=True)
                        nch += 1
                    if di % 2 == 0:
                        nc.scalar.copy(
                            rg[:, 0:DCH], ps[0:NCH, 0:DCH])
                        if nch == 2:
                            nc.scalar.copy(
                                rg[:, DCH:2 * DCH], ps[0:NCH, CH:CH + DCH])
                    else:
                        nc.vector.tensor_copy(rg[:, 0:DCH], ps[0:NCH, 0:DCH])
                        if nch == 2:
                            nc.vector.tensor_copy(
                                rg[:, DCH:2 * DCH], ps[0:NCH, CH:CH + DCH])
                    di += 1
                    r0 = half * HALFR + g0 * DROWS
                    dst = bass.AP(obuf.tensor, r0 * NCH * WO,
                                  [[WO, NCH], [NCH * WO, nch * DROWS], [1, WO]])
                    eng = nc.sync if (g0 // 2) % 2 == 0 else nc.scalar
                    eng.dma_start(dst, rg[:, 0:nch * DCH])

            # ---- col2im: one contiguous gather (reuses icol's SBUF slot),
            # kw-merge per kh in the free dim, then row-shift via 0/1
            # shift-matrix matmuls accumulating in PSUM.
            stall = big.tile([ROWS, NCH * WO], bf16, tag="icol")
            for kh in range(K):
                eng = (nc.sync, nc.scalar, nc.gpsimd)[kh % 3]
                eng.dma_start(
                    stall[:, kh * K * WO:(kh + 1) * K * WO],
                    bass.AP(obuf.tensor, kh * K * WO,
                            [[NCH * WO, ROWS], [1, K * WO]]))
            ops = pp.tile([128, 1024], f32, tag="ps")
            for kh in range(K):
                lhs = eshb[:, kh * SLAB:(kh + 1) * SLAB]
                for kw in range(K):
                    nc.tensor.matmul(
                        ops[0:SLAB, kw:kw + WO], lhs,
                        stall[:, (kh * K + kw) * WO:(kh * K + kw + 1) * WO],
                        start=(kh == 0 and kw == 0), stop=False)
            nc.tensor.matmul(ops[0:SLAB, 0:192], r(bandb), rb_sb[:],
                             start=False, stop=True)
            nc.scalar.copy(acc[:], ops[0:SLAB, 0:Wimg])
            nc.sync.dma_start(out_d, acc[:])

    n = _split_multi_waits(nc, mybir)
    return nc


def _get_nc():
    if "nc" not in _STATE:
        _STATE["nc"] = _build(use_f32r=True)
    return _STATE["nc"]


def _make_in_maps(I, WA, WD, WW, lmbda):
    import ml_dtypes  # noqa: F401
    I = np.ascontiguousarray(np.asarray(I, np.float32))
    WA = np.asarray(WA, np.float32)
    WD = np.asarray(WD, np.float32)
    WW = np.asarray(WW, np.float32)
    lam = np.asarray(lmbda, np.float32).reshape(F)
    assert I.shape == (B, 1, H, Wimg)

    WAc = (WA - WA.mean(axis=1, keepdims=True)).astype(np.float32)  # [64,81]
    S = (np.eye(F, dtype=np.float32) - WA @ WD).astype(np.float32)  # [64,64]
    sbd = np.zeros((128, 128), np.float32)
    sbd[0:F, 0:F] = S.T
    sbd[F:128, F:128] = S.T
    id128 = np.eye(128, dtype=np.float32)
    wwb = np.zeros((128, 2 * NCH), np.float32)
    wwb[0:F, 0:NCH] = WW.T
    wwb[F:128, NCH:2 * NCH] = WW.T
    lam128 = np.concatenate([lam, lam]).reshape(128, 1).astype(np.float32)
    esh = np.zeros((ROWS, K * SLAB), np.float32)  # lhsT per kh: E[r, y]=1 iff y=r+kh
    for kh in range(K):
        for rr in range(ROWS):
            esh[rr, kh * SLAB + rr + kh] = 1.0
    bandb = np.zeros((ROWS, SLAB), np.float32)
    for r in range(ROWS):
        for y in range(SLAB):
            if 0 <= y - r <= K - 1:
                bandb[r, y] = 1.0
    vals = {"wac": WAc.T, "sbd": sbd, "id128": id128,
            "wwb": wwb, "eshb": esh, "bandb": bandb}
    blob = np.zeros((128, BLOBC), np.float32)
    col = 0
    for name, np_, nf in BLOB_SPEC:
        v = np.asarray(vals[name], np.float32)
        assert v.shape == (np_, nf), (name, v.shape)
        blob[0:np_, col:col + nf] = v
        col += nf
    lams = np.concatenate([lam128, -lam128], axis=1).astype(np.float32)

    shared = {"blob": blob.astype(ml_dtypes.bfloat16), "lams": lams}
    in_maps = []
    for core in range(N_CORES):
        b, h = core // 2, core % 2
        r0 = h * ROWS
        slab = I[b, 0, r0:r0 + SLAB, :].astype(ml_dtypes.bfloat16).astype(
            np.float32)
        imgw = np.stack([slab[:, kw:kw + WO] for kw in range(K)], axis=0)
        # host-side mean prefix: rbm[r, x] = sum_kw mean[r, x-kw] with
        # mean = forward-box9(slab)/81 (the device adds the column-band sum
        # in the col2im PSUM via the bandb matmul)
        rs = np.zeros((SLAB, HO), np.float32)
        for kw in range(K):
            rs += slab[:, kw:kw + HO]
        mean = np.zeros((ROWS, HO), np.float32)
        for kh in range(K):
            mean += rs[kh:kh + ROWS, :]
        mean = (mean / NCH).astype(ml_dtypes.bfloat16).astype(np.float32)
        rbm = np.zeros((ROWS, 192), np.float32)
        for kw in range(K):
            rbm[:, kw:kw + HO] += mean
        in_maps.append({"imgw": np.ascontiguousarray(imgw).reshape(-1).astype(
            ml_dtypes.bfloat16),
            "rbm": rbm.astype(ml_dtypes.bfloat16), **shared})
    return in_maps


def _unshard(results):
    cnt = np.zeros((H, Wimg), np.float32)
    for kh in range(K):
        for kw in range(K):
            cnt[kh:kh + HO, kw:kw + WO] += 1.0
    out = np.zeros((B, 1, H, Wimg), np.float32)
    for b in range(B):
        acc = np.zeros((H, Wimg), np.float32)
        acc[0:SLAB, :] += results[2 * b]["out"]
        acc[ROWS:ROWS + SLAB, :] += results[2 * b + 1]["out"]
        out[b, 0] = acc / cnt
    return out


def kernel(I, WA, WD, WW, lmbda, kernel_size=9, stride=1, unfoldings=24, **_kw):
    from concourse import bass_utils

    assert int(kernel_size) == K and int(stride) == 1 and int(unfoldings) == UNF
    in_maps = _make_in_maps(I, WA, WD, WW, lmbda)
    nc = _get_nc()
    last = None
    for _attempt in range(3):
        try:
            res = bass_utils.run_bass_kernel_spmd(
                nc, in_maps, core_ids=list(range(N_CORES)))
            return _unshard(res.results)
        except Exception as e:  # transient NRT device errors: retry
            last = e
    raise last



# revision 20
# speedup vs baseline: 1.0376x; 1.0012x over previous
"""LISTA (learned ISTA) sparse-coding forward pass on 8 Trainium2 NeuronCores.

Problem: I [4,1,192,192] -> im2col(9x9) -> 24 soft-thresholded iterations over
64 filters -> decode -> col2im overlap-add average -> [4,1,192,192].

Sharding: 8 cores = 4 images x 2 position-row halves (92 rows of 184 positions
each). Each core computes its full LISTA pipeline plus the col2im partial sums
for its 100-row output slab; the host merges the 8-row seams between the two
slabs of each image and divides by the overlap counts (pure unshard glue).

Algebra used (exact rewrites of the reference up to fp assoc.):
  - mean-subtraction folded into encoder:  c = WAc @ I_col,
      WAc = WA - rowmean(WA)  (since mean_patch = (1/81) * ones^T I_col)
  - iteration fused:  gamma_{t+1} = soft(S @ gamma_t + c),  S = I - WA@WD
  - gamma kept as a + bneg with a = relu(y-l) >= 0, bneg = min(y+l, 0) <= 0
    (soft(y) = a + bneg), so the subtraction never needs its own pass:
      y_{t+1} = Id@c + S@a_t + S@bneg_t   (3 accumulating PE passes)
  - decode: out_all = WW@a + WW@bneg + (J/81) @ I_col   (mean add-back)
"""

import contextlib
import numpy as np

# ---------------------------------------------------------------- constants
B, H, Wimg = 4, 192, 192
K = 9
F = 64
NCH = K * K  # 81
HO = H - K + 1  # 184
WO = Wimg - K + 1  # 184
UNF = 24
N_CORES = 8

ROWS = HO // 2  # 92 position rows per core
SLAB = ROWS + K - 1  # 100 image/output rows per core
NPOS = ROWS * WO  # 16928 positions per core
HALFR = ROWS // 2  # 46 rows per block-diag half
HALF = HALFR * WO  # 8464 columns per half

CH = 512
CHUNKS = [(i * CH, min((i + 1) * CH, HALF)) for i in range((HALF + CH - 1) // CH)]
SUPERS = [CHUNKS[i : i + 2] for i in range(0, len(CHUNKS), 2)]
DROWS = 2  # decode chunk = 2 position rows
DCH = DROWS * WO  # 368 columns

# weight blob layout: (name, partitions, cols) — bf16
BLOB_SPEC = [
    ("wac", NCH, F), ("sbd", 128, 128), ("id128", 128, 128),
    ("wwb", 128, 2 * NCH), ("eshb", ROWS, K * SLAB),
    ("bandb", ROWS, SLAB),
]
BLOBC = sum(nf for _, _, nf in BLOB_SPEC)

_STATE = {}


def _split_multi_waits(nc, mybir):
    """This walrus build supports a single sync-wait slot per instruction.
    Move extra waits onto preceding same-engine no-ops (same semantics:
    program order on one engine; all waits clear before the instruction)."""
    cnt = 0
    for fn in nc.m.functions:
        for bb in fn.blocks:
            insts = bb.instructions
            need = False
            for ins in insts:
                si = ins.sync_info
                if si is not None and si.on_wait is not None and len(si.on_wait) > 1:
                    need = True
                    break
            if not need:
                continue
            out = []
            for ins in insts:
                si = ins.sync_info
                if si is not None and si.on_wait is not None and len(si.on_wait) > 1:
                    waits = list(si.on_wait)
                    for w in waits[:-1]:
                        cnt += 1
                        nop = mybir.InstNoOp(name=f"wsplit-{cnt}", ins=[], outs=[])
                        nop.engine = ins.engine
                        nop.sync_info = mybir.SyncInfo(on_wait=[w], on_update=[])
                        out.append(nop)
                    ins.sync_info = mybir.SyncInfo(
                        on_wait=[waits[-1]], on_update=list(si.on_update or [])
                    )
                out.append(ins)
            bb.instructions = out
    return cnt


def _build(use_f32r=True):
    import concourse.bass as bass
    import concourse.mybir as mybir
    import concourse.tile as tile

    f32 = mybir.dt.float32
    bf16 = mybir.dt.bfloat16
    f32r = mybir.dt.float32r
    Alu = mybir.AluOpType
    Act = mybir.ActivationFunctionType

    nc = bass.Bass("TRN2", target_bir_lowering=False, debug=False)

    mmdt = bf16

    imgw = nc.dram_tensor("imgw", [K * SLAB * WO], mmdt, kind="ExternalInput").ap()
    blob_d = nc.dram_tensor("blob", [128, BLOBC], mmdt, kind="ExternalInput").ap()
    lams_d = nc.dram_tensor("lams", [128, 2], f32, kind="ExternalInput").ap()
    rbm_d = nc.dram_tensor("rbm", [ROWS, 192], bf16, kind="ExternalInput").ap()
    out_d = nc.dram_tensor("out", [SLAB, Wimg], f32, kind="ExternalOutput").ap()
    obuf = nc.dram_tensor("obuf", [NCH * NPOS], bf16, kind="Internal").ap()

    def r(ap):
        return ap

    with tile.TileContext(nc) as tc:
        with contextlib.ExitStack() as ctx:
            wpool = ctx.enter_context(tc.tile_pool(name="w", bufs=1))
            big = ctx.enter_context(tc.tile_pool(name="big", bufs=1))
            pp = ctx.enter_context(tc.tile_pool(name="ps", bufs=4, space="PSUM"))
            ring = ctx.enter_context(tc.tile_pool(name="ring", bufs=4))
            stg = ctx.enter_context(tc.tile_pool(name="stg", bufs=2))

            blob = wpool.tile([128, BLOBC], mmdt)
            nc.gpsimd.dma_start(blob[:], blob_d)
            o = {}
            col = 0
            for name, np_, nf in BLOB_SPEC:
                o[name] = (np_, col, nf)
                col += nf
            def bl(name, cast=None):
                np_, c0, nf = o[name]
                v = blob[0:np_, c0:c0 + nf]
                return v.bitcast(f32) if cast else v
            wac = bl("wac"); sbd = bl("sbd")
            id128 = bl("id128"); wwb = bl("wwb")
            eshb = bl("eshb"); bandb = bl("bandb")
            lams = wpool.tile([128, 2], f32)
            nc.gpsimd.dma_start(lams[:], lams_d)
            lam = lams[:, 0:1]
            nlam = lams[:, 1:2]

            icol = big.tile([NCH, NPOS], mmdt, tag="icol")
            rb_sb = wpool.tile([ROWS, 192], mmdt)
            c = big.tile([128, HALF], mmdt)
            gam = big.tile([128, HALF], mmdt)
            acc = big.tile([SLAB, Wimg], f32)
            ypool = ctx.enter_context(tc.tile_pool(name="y", bufs=4))

            # ---- im2col: host supplies img_w[kw] = slab[:, kw:kw+WO]; each
            # channel (kh, kw) = img_w[kw][kh:kh+ROWS] is one contiguous run.
            ebnds = [0, 12, 24, 35, 46]
            for hh in range(2):
                eng = nc.sync if hh == 0 else nc.scalar
                for q in range(4):
                    r0 = hh * HALFR + ebnds[q]
                    r1 = hh * HALFR + ebnds[q + 1]
                    eng.dma_start(
                        icol[:, r0 * WO:r1 * WO],
                        bass.AP(imgw.tensor, r0 * WO,
                                [[WO, K], [SLAB * WO, K], [1, (r1 - r0) * WO]]))
            nc.scalar.dma_start(rb_sb[:], rbm_d)
            # ---- encode: c = WAc @ I_col, col-tiled concurrent pair
            # (h1 -> psum[0:64], h2 -> psum[64:128]); one bf16 copy, then
            # gamma0 = soft(c2) from the copy (Pool clip + DVE sub).
            for si, sup in enumerate(SUPERS):
                ps = pp.tile([128, 1024], f32, tag="ps")
                c0s, c1s = sup[0][0], sup[-1][1]
                for jj, (c0, c1) in enumerate(sup):
                    n = c1 - c0
                    nc.tensor.matmul(ps[0:F, jj * CH: jj * CH + n], r(wac),
                                     r(icol[:, c0:c1]), start=True, stop=True)
                    nc.tensor.matmul(ps[F:128, jj * CH: jj * CH + n], r(wac),
                                     r(icol[:, HALF + c0:HALF + c1]),
                                     start=True, stop=True)
                span = c1s - c0s
                nc.scalar.copy(c[:, c0s:c1s], ps[:, 0:span])
                z = ypool.tile([128, 1024], mmdt, tag="z0")
                nc.vector.tensor_scalar(z[:, 0:span], c[:, c0s:c1s],
                                        lam, nlam, Alu.min, Alu.max)
                nc.vector.tensor_tensor(gam[:, c0s:c1s], c[:, c0s:c1s],
                                        z[:, 0:span], Alu.subtract)

            # ---- 23 fused iterations: y = Id@c + S@gam; gam' = y - clip(y)
            # ACT copies y out per 1024-superstep; DVE clip+sub batched in
            # 2048 pairs to amortize per-op overheads.
            for _t in range(UNF - 1):
                si = 0
                for sp in range(0, len(SUPERS), 2):
                    pair = SUPERS[sp:sp + 2]
                    p0 = pair[0][0][0]
                    yt = None
                    if any(s[-1][1] - s[0][0] >= CH for s in pair):
                        yt = ypool.tile([128, 2048], mmdt, tag="y")
                    off = 0
                    tails = []
                    for sup in pair:
                        ps = pp.tile([128, 1024], f32, tag="ps")
                        c0s, c1s = sup[0][0], sup[-1][1]
                        for jj, (c0, c1) in enumerate(sup):
                            nc.tensor.matmul(ps[:, jj * CH: jj * CH + (c1 - c0)],
                                             r(id128), r(c[:, c0:c1]),
                                             start=True, stop=False)
                        for jj, (c0, c1) in enumerate(sup):
                            nc.tensor.matmul(ps[:, jj * CH: jj * CH + (c1 - c0)],
                                             r(sbd), r(gam[:, c0:c1]),
                                             start=False, stop=True)
                        span = c1s - c0s
                        if span < CH:
                            # tail superstep: DVE clip+sub straight from PSUM
                            tails.append((ps, c0s, span))
                        else:
                            nc.scalar.copy(yt[:, off:off + span], ps[:, 0:span])
                            off += span
                    if off:
                        z = ypool.tile([128, 2048], mmdt, tag="z")
                        nc.vector.tensor_scalar(z[:, 0:off], yt[:, 0:off],
                                                lam, nlam, Alu.min, Alu.max)
                        nc.vector.tensor_tensor(gam[:, p0:p0 + off],
                                                yt[:, 0:off],
                                                z[:, 0:off], Alu.subtract)
                    for (ps, c0s, span) in tails:
                        zt = ypool.tile([128, 2048], mmdt, tag="z")
                        nc.vector.tensor_scalar(zt[:, 0:span], ps[:, 0:span],
                                                lam, nlam, Alu.min, Alu.max)
                        nc.vector.tensor_tensor(gam[:, c0s:c0s + span],
                                                ps[:, 0:span],
                                                zt[:, 0:span], Alu.subtract)

            # ---- decode: out_all = WW@gam only (mean term handled by the
            # bm path in col2im), stream to HBM row-major
            ngrp = HALFR // DROWS  # 23 two-row chunks per half
            di = 0
            for half in range(2):
                ww = wwb[:, 0:NCH] if half == 0 else wwb[:, NCH:2 * NCH]
                for g0 in range(0, ngrp, 2):
                    ps = pp.tile([128, 1024], f32, tag="ps")
                    rg = ring.tile([NCH, 2 * DCH], bf16, tag="ring")
                    nch = 0
                    for jj, g in enumerate(range(g0, min(g0 + 2, ngrp))):
                        c0 = g * DCH
                        sl = ps[0:NCH, jj * CH: jj * CH + DCH]
                        nc.tensor.matmul(sl, r(ww),
                                         r(gam[:, c0:c0 + DCH]),
                                         start=True, stop=True)
                        nch += 1
                    nc.scalar.copy(rg[:, 0:DCH], ps[0:NCH, 0:DCH])
                    if nch == 2:
                        nc.vector.tensor_copy(
                            rg[:, DCH:2 * DCH], ps[0:NCH, CH:CH + DCH])
                    di += 1
                    r0 = half * HALFR + g0 * DROWS
                    dst = bass.AP(obuf.tensor, r0 * NCH * WO,
                                  [[WO, NCH], [NCH * WO, nch * DROWS], [1, WO]])
                    eng = nc.sync if (g0 // 2) % 2 == 0 else nc.scalar
                    eng.dma_start(dst, rg[:, 0:nch * DCH])

            # ---- col2im: one contiguous gather (reuses icol's SBUF slot),
            # kw-merge per kh in the free dim, then row-shift via 0/1
            # shift-matrix matmuls accumulating in PSUM.
            stall = big.tile([ROWS, NCH * WO], bf16, tag="icol")
            for kh in range(K):
                eng = (nc.sync, nc.scalar, nc.gpsimd)[kh % 3]
                eng.dma_start(
                    stall[:, kh * K * WO:(kh + 1) * K * WO],
                    bass.AP(obuf.tensor, kh * K * WO,
                            [[NCH * WO, ROWS], [1, K * WO]]))
            ops = pp.tile([128, 1024], f32, tag="ps")
            for kh in range(K):
                lhs = eshb[:, kh * SLAB:(kh + 1) * SLAB]
                for kw in range(K):
                    nc.tensor.matmul(
                        ops[0:SLAB, kw:kw + WO], lhs,
                        stall[:, (kh * K + kw) * WO:(kh * K + kw + 1) * WO],
                        start=(kh == 0 and kw == 0), stop=False)
            nc.tensor.matmul(ops[0:SLAB, 0:192], r(bandb), rb_sb[:],
                             start=False, stop=True)
            nc.scalar.copy(acc[:], ops[0:SLAB, 0:Wimg])
            nc.sync.dma_start(out_d, acc[:])

    n = _split_multi_waits(nc, mybir)
    return nc


def _get_nc():
    if "nc" not in _STATE:
        _STATE["nc"] = _build(use_f32r=True)
    return _STATE["nc"]


def _make_in_maps(I, WA, WD, WW, lmbda):
    import ml_dtypes  # noqa: F401
    I = np.ascontiguousarray(np.asarray(I, np.float32))
    WA = np.asarray(WA, np.float32)
    WD = np.asarray(WD, np.float32)
    WW = np.asarray(WW, np.float32)
    lam = np.asarray(lmbda, np.float32).reshape(F)
    assert I.shape == (B, 1, H, Wimg)

    WAc = (WA - WA.mean(axis=1, keepdims=True)).astype(np.float32)  # [64,81]
    S = (np.eye(F, dtype=np.float32) - WA @ WD).astype(np.float32)  # [64,64]
    sbd = np.zeros((128, 128), np.float32)
    sbd[0:F, 0:F] = S.T
    sbd[F:128, F:128] = S.T
    id128 = np.eye(128, dtype=np.float32)
    wwb = np.zeros((128, 2 * NCH), np.float32)
    wwb[0:F, 0:NCH] = WW.T
    wwb[F:128, NCH:2 * NCH] = WW.T
    lam128 = np.concatenate([lam, lam]).reshape(128, 1).astype(np.float32)
    esh = np.zeros((ROWS, K * SLAB), np.float32)  # lhsT per kh: E[r, y]=1 iff y=r+kh
    for kh in range(K):
        for rr in range(ROWS):
            esh[rr, kh * SLAB + rr + kh] = 1.0
    bandb = np.zeros((ROWS, SLAB), np.float32)
    for r in range(ROWS):
        for y in range(SLAB):
            if 0 <= y - r <= K - 1:
                bandb[r, y] = 1.0
    vals = {"wac": WAc.T, "sbd": sbd, "id128": id128,
            "wwb": wwb, "eshb": esh, "bandb": bandb}
    blob = np.zeros((128, BLOBC), np.float32)
    col = 0
    for name, np_, nf in BLOB_SPEC:
        v = np.asarray(vals[name], np.float32)
        assert v.shape == (np_, nf), (name, v.shape)
        blob[0:np_, col:col + nf] = v
        col += nf
    lams = np.concatenate([lam128, -lam128], axis=1).astype(np.float32)

    shared = {"blob": blob.astype(ml_dtypes.bfloat16), "lams": lams}
    in_maps = []
    for core in range(N_CORES):
        b, h = core // 2, core % 2
        r0 = h * ROWS
        slab = I[b, 0, r0:r0 + SLAB, :].astype(ml_dtypes.bfloat16).astype(
            np.float32)
        imgw = np.stack([slab[:, kw:kw + WO] for kw in range(K)], axis=0)
        # host-side mean prefix: rbm[r, x] = sum_kw mean[r, x-kw] with
        # mean = forward-box9(slab)/81 (the device adds the column-band sum
        # in the col2im PSUM via the bandb matmul)
        rs = np.zeros((SLAB, HO), np.float32)
        for kw in range(K):
            rs += slab[:, kw:kw + HO]
        mean = np.zeros((ROWS, HO), np.float32)
        for kh in range(K):
            mean += rs[kh:kh + ROWS, :]
        mean = (mean / NCH).astype(ml_dtypes.bfloat16).astype(np.float32)
        rbm = np.zeros((ROWS, 192), np.float32)
        for kw in range(K):
            rbm[:, kw:kw + HO] += mean
        in_maps.append({"imgw": np.ascontiguousarray(imgw).reshape(-1).astype(
            ml_dtypes.bfloat16),
            "rbm": rbm.astype(ml_dtypes.bfloat16), **shared})
    return in_maps


def _unshard(results):
    cnt = np.zeros((H, Wimg), np.float32)
    for kh in range(K):
        for kw in range(K):
            cnt[kh:kh + HO, kw:kw + WO] += 1.0
    out = np.zeros((B, 1, H, Wimg), np.float32)
    for b in range(B):
        acc = np.zeros((H, Wimg), np.float32)
        acc[0:SLAB, :] += results[2 * b]["out"]
        acc[ROWS:ROWS + SLAB, :] += results[2 * b + 1]["out"]
        out[b, 0] = acc / cnt
    return out


def kernel(I, WA, WD, WW, lmbda, kernel_size=9, stride=1, unfoldings=24, **_kw):
    from concourse import bass_utils

    assert int(kernel_size) == K and int(stride) == 1 and int(unfoldings) == UNF
    in_maps = _make_in_maps(I, WA, WD, WW, lmbda)
    nc = _get_nc()
    last = None
    for _attempt in range(3):
        try:
            res = bass_utils.run_bass_kernel_spmd(
                nc, in_maps, core_ids=list(range(N_CORES)))
            return _unshard(res.results)
        except Exception as e:  # transient NRT device errors: retry
            last = e
    raise last



# revision 21
# speedup vs baseline: 1.0694x; 1.0306x over previous
"""LISTA (learned ISTA) sparse-coding forward pass on 8 Trainium2 NeuronCores.

Problem: I [4,1,192,192] -> im2col(9x9) -> 24 soft-thresholded iterations over
64 filters -> decode -> col2im overlap-add average -> [4,1,192,192].

Sharding: 8 cores = 4 images x 2 position-row halves (92 rows of 184 positions
each). Each core computes its full LISTA pipeline plus the col2im partial sums
for its 100-row output slab; the host merges the 8-row seams between the two
slabs of each image and divides by the overlap counts (pure unshard glue).

Algebra used (exact rewrites of the reference up to fp assoc.):
  - mean-subtraction folded into encoder:  c = WAc @ I_col,
      WAc = WA - rowmean(WA)  (since mean_patch = (1/81) * ones^T I_col)
  - iteration fused:  gamma_{t+1} = soft(S @ gamma_t + c),  S = I - WA@WD
  - gamma kept as a + bneg with a = relu(y-l) >= 0, bneg = min(y+l, 0) <= 0
    (soft(y) = a + bneg), so the subtraction never needs its own pass:
      y_{t+1} = Id@c + S@a_t + S@bneg_t   (3 accumulating PE passes)
  - decode: out_all = WW@a + WW@bneg + (J/81) @ I_col   (mean add-back)
"""

import contextlib
import numpy as np

# ---------------------------------------------------------------- constants
B, H, Wimg = 4, 192, 192
K = 9
F = 64
NCH = K * K  # 81
HO = H - K + 1  # 184
WO = Wimg - K + 1  # 184
UNF = 24
N_CORES = 8

ROWS = HO // 2  # 92 position rows per core
SLAB = ROWS + K - 1  # 100 image/output rows per core
NPOS = ROWS * WO  # 16928 positions per core
HALFR = ROWS // 2  # 46 rows per block-diag half
HALF = HALFR * WO  # 8464 columns per half

CH = 512
CHUNKS = [(i * CH, min((i + 1) * CH, HALF)) for i in range((HALF + CH - 1) // CH)]
SUPERS = [CHUNKS[i : i + 2] for i in range(0, len(CHUNKS), 2)]
DROWS = 2  # decode chunk = 2 position rows
DCH = DROWS * WO  # 368 columns

# weight blob layout: (name, partitions, cols) — bf16
BLOB_SPEC = [
    ("wac", NCH, F), ("sbd", 128, 128), ("id128", 128, 128),
    ("wwb", 128, 2 * NCH), ("eshb", ROWS, K * SLAB),
    ("bandb", ROWS, SLAB),
]
BLOBC = sum(nf for _, _, nf in BLOB_SPEC)

_STATE = {}


def _split_multi_waits(nc, mybir):
    """This walrus build supports a single sync-wait slot per instruction.
    Move extra waits onto preceding same-engine no-ops (same semantics:
    program order on one engine; all waits clear before the instruction)."""
    cnt = 0
    for fn in nc.m.functions:
        for bb in fn.blocks:
            insts = bb.instructions
            need = False
            for ins in insts:
                si = ins.sync_info
                if si is not None and si.on_wait is not None and len(si.on_wait) > 1:
                    need = True
                    break
            if not need:
                continue
            out = []
            for ins in insts:
                si = ins.sync_info
                if si is not None and si.on_wait is not None and len(si.on_wait) > 1:
                    waits = list(si.on_wait)
                    for w in waits[:-1]:
                        cnt += 1
                        nop = mybir.InstNoOp(name=f"wsplit-{cnt}", ins=[], outs=[])
                        nop.engine = ins.engine
                        nop.sync_info = mybir.SyncInfo(on_wait=[w], on_update=[])
                        out.append(nop)
                    ins.sync_info = mybir.SyncInfo(
                        on_wait=[waits[-1]], on_update=list(si.on_update or [])
                    )
                out.append(ins)
            bb.instructions = out
    return cnt


def _build(use_f32r=True):
    import concourse.bass as bass
    import concourse.mybir as mybir
    import concourse.tile as tile

    f32 = mybir.dt.float32
    bf16 = mybir.dt.bfloat16
    f32r = mybir.dt.float32r
    Alu = mybir.AluOpType
    Act = mybir.ActivationFunctionType

    nc = bass.Bass("TRN2", target_bir_lowering=False, debug=False)

    mmdt = bf16

    imgw = nc.dram_tensor("imgw", [K * SLAB * WO], mmdt, kind="ExternalInput").ap()
    blob_d = nc.dram_tensor("blob", [128, BLOBC], mmdt, kind="ExternalInput").ap()
    lams_d = nc.dram_tensor("lams", [128, 2], f32, kind="ExternalInput").ap()
    rbm_d = nc.dram_tensor("rbm", [ROWS, 192], bf16, kind="ExternalInput").ap()
    out_d = nc.dram_tensor("out", [SLAB, Wimg], f32, kind="ExternalOutput").ap()
    obuf = nc.dram_tensor("obuf", [NCH * NPOS], bf16, kind="Internal").ap()

    def r(ap):
        return ap

    with tile.TileContext(nc) as tc:
        with contextlib.ExitStack() as ctx:
            wpool = ctx.enter_context(tc.tile_pool(name="w", bufs=1))
            big = ctx.enter_context(tc.tile_pool(name="big", bufs=1))
            pp = ctx.enter_context(tc.tile_pool(name="ps", bufs=4, space="PSUM"))
            ring = ctx.enter_context(tc.tile_pool(name="ring", bufs=6))
            stg = ctx.enter_context(tc.tile_pool(name="stg", bufs=2))

            blob = wpool.tile([128, BLOBC], mmdt)
            nc.gpsimd.dma_start(blob[:], blob_d)
            o = {}
            col = 0
            for name, np_, nf in BLOB_SPEC:
                o[name] = (np_, col, nf)
                col += nf
            def bl(name, cast=None):
                np_, c0, nf = o[name]
                v = blob[0:np_, c0:c0 + nf]
                return v.bitcast(f32) if cast else v
            wac = bl("wac"); sbd = bl("sbd")
            id128 = bl("id128"); wwb = bl("wwb")
            eshb = bl("eshb"); bandb = bl("bandb")
            lams = wpool.tile([128, 2], f32)
            nc.gpsimd.dma_start(lams[:], lams_d)
            lam = lams[:, 0:1]
            nlam = lams[:, 1:2]

            icol = big.tile([NCH, NPOS], mmdt, tag="icol")
            rb_sb = wpool.tile([ROWS, 192], mmdt)
            c = big.tile([128, HALF], mmdt)
            gam = big.tile([128, HALF], mmdt)
            acc = big.tile([SLAB, Wimg], f32)
            ypool = ctx.enter_context(tc.tile_pool(name="y", bufs=4))

            # ---- im2col: host supplies img_w[kw] = slab[:, kw:kw+WO]; each
            # channel (kh, kw) = img_w[kw][kh:kh+ROWS] is one contiguous run.
            ebnds = [0, 12, 24, 35, 46]
            for hh in range(2):
                eng = nc.sync if hh == 0 else nc.scalar
                for q in range(4):
                    r0 = hh * HALFR + ebnds[q]
                    r1 = hh * HALFR + ebnds[q + 1]
                    eng.dma_start(
                        icol[:, r0 * WO:r1 * WO],
                        bass.AP(imgw.tensor, r0 * WO,
                                [[WO, K], [SLAB * WO, K], [1, (r1 - r0) * WO]]))
            nc.scalar.dma_start(rb_sb[:], rbm_d)
            # ---- encode: c = WAc @ I_col, col-tiled concurrent pair
            # (h1 -> psum[0:64], h2 -> psum[64:128]); one bf16 copy, then
            # gamma0 = soft(c2) from the copy (Pool clip + DVE sub).
            for si, sup in enumerate(SUPERS):
                ps = pp.tile([128, 1024], f32, tag="ps")
                c0s, c1s = sup[0][0], sup[-1][1]
                for jj, (c0, c1) in enumerate(sup):
                    n = c1 - c0
                    nc.tensor.matmul(ps[0:F, jj * CH: jj * CH + n], r(wac),
                                     r(icol[:, c0:c1]), start=True, stop=True)
                    nc.tensor.matmul(ps[F:128, jj * CH: jj * CH + n], r(wac),
                                     r(icol[:, HALF + c0:HALF + c1]),
                                     start=True, stop=True)
                span = c1s - c0s
                nc.scalar.copy(c[:, c0s:c1s], ps[:, 0:span])
                z = ypool.tile([128, 1024], mmdt, tag="z0")
                nc.vector.tensor_scalar(z[:, 0:span], c[:, c0s:c1s],
                                        lam, nlam, Alu.min, Alu.max)
                nc.vector.tensor_tensor(gam[:, c0s:c1s], c[:, c0s:c1s],
                                        z[:, 0:span], Alu.subtract)

            # ---- 23 fused iterations: y = Id@c + S@gam; gam' = y - clip(y)
            # ACT copies y out per 1024-superstep; DVE clip+sub batched in
            # 2048 pairs to amortize per-op overheads.
            for _t in range(UNF - 1):
                si = 0
                for sp in range(0, len(SUPERS), 2):
                    pair = SUPERS[sp:sp + 2]
                    p0 = pair[0][0][0]
                    yt = None
                    if any(s[-1][1] - s[0][0] >= CH for s in pair):
                        yt = ypool.tile([128, 2048], mmdt, tag="y")
                    off = 0
                    tails = []
                    for sup in pair:
                        ps = pp.tile([128, 1024], f32, tag="ps")
                        c0s, c1s = sup[0][0], sup[-1][1]
                        for jj, (c0, c1) in enumerate(sup):
                            nc.tensor.matmul(ps[:, jj * CH: jj * CH + (c1 - c0)],
                                             r(id128), r(c[:, c0:c1]),
                                             start=True, stop=False)
                        for jj, (c0, c1) in enumerate(sup):
                            nc.tensor.matmul(ps[:, jj * CH: jj * CH + (c1 - c0)],
                                             r(sbd), r(gam[:, c0:c1]),
                                             start=False, stop=True)
                        span = c1s - c0s
                        if span < CH:
                            # tail superstep: DVE clip+sub straight from PSUM
                            tails.append((ps, c0s, span))
                        else:
                            nc.scalar.copy(yt[:, off:off + span], ps[:, 0:span])
                            off += span
                    if off:
                        z = ypool.tile([128, 2048], mmdt, tag="z")
                        nc.vector.tensor_scalar(z[:, 0:off], yt[:, 0:off],
                                                lam, nlam, Alu.min, Alu.max)
                        nc.vector.tensor_tensor(gam[:, p0:p0 + off],
                                                yt[:, 0:off],
                                                z[:, 0:off], Alu.subtract)
                    for (ps, c0s, span) in tails:
                        zt = ypool.tile([128, 2048], mmdt, tag="z")
                        nc.vector.tensor_scalar(zt[:, 0:span], ps[:, 0:span],
                                                lam, nlam, Alu.min, Alu.max)
                        nc.vector.tensor_tensor(gam[:, c0s:c0s + span],
                                                ps[:, 0:span],
                                                zt[:, 0:span], Alu.subtract)

            # ---- decode: out_all = WW@gam only (mean term handled by the
            # bm path in col2im), stream to HBM row-major
            ngrp = HALFR // DROWS  # 23 two-row chunks per half
            di = 0
            for half in range(2):
                ww = wwb[:, 0:NCH] if half == 0 else wwb[:, NCH:2 * NCH]
                for g0 in range(0, ngrp, 2):
                    ps = pp.tile([128, 1024], f32, tag="ps")
                    rg = ring.tile([NCH, 2 * DCH], bf16, tag="ring")
                    nch = 0
                    for jj, g in enumerate(range(g0, min(g0 + 2, ngrp))):
                        c0 = g * DCH
                        sl = ps[0:NCH, jj * CH: jj * CH + DCH]
                        nc.tensor.matmul(sl, r(ww),
                                         r(gam[:, c0:c0 + DCH]),
                                         start=True, stop=True)
                        nch += 1
                    nc.scalar.copy(rg[:, 0:DCH], ps[0:NCH, 0:DCH])
                    if nch == 2:
                        nc.vector.tensor_copy(
                            rg[:, DCH:2 * DCH], ps[0:NCH, CH:CH + DCH])
                    di += 1
                    r0 = half * HALFR + g0 * DROWS
                    dst = bass.AP(obuf.tensor, r0 * NCH * WO,
                                  [[WO, NCH], [NCH * WO, nch * DROWS], [1, WO]])
                    nc.sync.dma_start(dst, rg[:, 0:nch * DCH])

            # ---- col2im: one contiguous gather (reuses icol's SBUF slot),
            # kw-merge per kh in the free dim, then row-shift via 0/1
            # shift-matrix matmuls accumulating in PSUM.
            stall = big.tile([ROWS, NCH * WO], bf16, tag="icol")
            for kh in range(K):
                eng = (nc.gpsimd, nc.sync, nc.gpsimd)[kh % 3]
                eng.dma_start(
                    stall[:, kh * K * WO:(kh + 1) * K * WO],
                    bass.AP(obuf.tensor, kh * K * WO,
                            [[NCH * WO, ROWS], [1, K * WO]]))
            ops = pp.tile([128, 1024], f32, tag="ps")
            for kh in range(K):
                lhs = eshb[:, kh * SLAB:(kh + 1) * SLAB]
                for kw in range(K):
                    nc.tensor.matmul(
                        ops[0:SLAB, kw:kw + WO], lhs,
                        stall[:, (kh * K + kw) * WO:(kh * K + kw + 1) * WO],
                        start=(kh == 0 and kw == 0), stop=False)
            nc.tensor.matmul(ops[0:SLAB, 0:192], r(bandb), rb_sb[:],
                             start=False, stop=True)
            nc.scalar.copy(acc[:], ops[0:SLAB, 0:Wimg])
            nc.sync.dma_start(out_d, acc[:])

    n = _split_multi_waits(nc, mybir)
    return nc


def _get_nc():
    if "nc" not in _STATE:
        _STATE["nc"] = _build(use_f32r=True)
    return _STATE["nc"]


def _make_in_maps(I, WA, WD, WW, lmbda):
    import ml_dtypes  # noqa: F401
    I = np.ascontiguousarray(np.asarray(I, np.float32))
    WA = np.asarray(WA, np.float32)
    WD = np.asarray(WD, np.float32)
    WW = np.asarray(WW, np.float32)
    lam = np.asarray(lmbda, np.float32).reshape(F)
    assert I.shape == (B, 1, H, Wimg)

    WAc = (WA - WA.mean(axis=1, keepdims=True)).astype(np.float32)  # [64,81]
    S = (np.eye(F, dtype=np.float32) - WA @ WD).astype(np.float32)  # [64,64]
    sbd = np.zeros((128, 128), np.float32)
    sbd[0:F, 0:F] = S.T
    sbd[F:128, F:128] = S.T
    id128 = np.eye(128, dtype=np.float32)
    wwb = np.zeros((128, 2 * NCH), np.float32)
    wwb[0:F, 0:NCH] = WW.T
    wwb[F:128, NCH:2 * NCH] = WW.T
    lam128 = np.concatenate([lam, lam]).reshape(128, 1).astype(np.float32)
    esh = np.zeros((ROWS, K * SLAB), np.float32)  # lhsT per kh: E[r, y]=1 iff y=r+kh
    for kh in range(K):
        for rr in range(ROWS):
            esh[rr, kh * SLAB + rr + kh] = 1.0
    bandb = np.zeros((ROWS, SLAB), np.float32)
    for r in range(ROWS):
        for y in range(SLAB):
            if 0 <= y - r <= K - 1:
                bandb[r, y] = 1.0
    vals = {"wac": WAc.T, "sbd": sbd, "id128": id128,
            "wwb": wwb, "eshb": esh, "bandb": bandb}
    blob = np.zeros((128, BLOBC), np.float32)
    col = 0
    for name, np_, nf in BLOB_SPEC:
        v = np.asarray(vals[name], np.float32)
        assert v.shape == (np_, nf), (name, v.shape)
        blob[0:np_, col:col + nf] = v
        col += nf
    lams = np.concatenate([lam128, -lam128], axis=1).astype(np.float32)

    shared = {"blob": blob.astype(ml_dtypes.bfloat16), "lams": lams}
    in_maps = []
    for core in range(N_CORES):
        b, h = core // 2, core % 2
        r0 = h * ROWS
        slab = I[b, 0, r0:r0 + SLAB, :].astype(ml_dtypes.bfloat16).astype(
            np.float32)
        imgw = np.stack([slab[:, kw:kw + WO] for kw in range(K)], axis=0)
        # host-side mean prefix: rbm[r, x] = sum_kw mean[r, x-kw] with
        # mean = forward-box9(slab)/81 (the device adds the column-band sum
        # in the col2im PSUM via the bandb matmul)
        rs = np.zeros((SLAB, HO), np.float32)
        for kw in range(K):
            rs += slab[:, kw:kw + HO]
        mean = np.zeros((ROWS, HO), np.float32)
        for kh in range(K):
            mean += rs[kh:kh + ROWS, :]
        mean = (mean / NCH).astype(ml_dtypes.bfloat16).astype(np.float32)
        rbm = np.zeros((ROWS, 192), np.float32)
        for kw in range(K):
            rbm[:, kw:kw + HO] += mean
        in_maps.append({"imgw": np.ascontiguousarray(imgw).reshape(-1).astype(
            ml_dtypes.bfloat16),
            "rbm": rbm.astype(ml_dtypes.bfloat16), **shared})
    return in_maps


def _unshard(results):
    cnt = np.zeros((H, Wimg), np.float32)
    for kh in range(K):
        for kw in range(K):
            cnt[kh:kh + HO, kw:kw + WO] += 1.0
    out = np.zeros((B, 1, H, Wimg), np.float32)
    for b in range(B):
        acc = np.zeros((H, Wimg), np.float32)
        acc[0:SLAB, :] += results[2 * b]["out"]
        acc[ROWS:ROWS + SLAB, :] += results[2 * b + 1]["out"]
        out[b, 0] = acc / cnt
    return out


def kernel(I, WA, WD, WW, lmbda, kernel_size=9, stride=1, unfoldings=24, **_kw):
    from concourse import bass_utils

    assert int(kernel_size) == K and int(stride) == 1 and int(unfoldings) == UNF
    in_maps = _make_in_maps(I, WA, WD, WW, lmbda)
    nc = _get_nc()
    last = None
    for _attempt in range(3):
        try:
            res = bass_utils.run_bass_kernel_spmd(
                nc, in_maps, core_ids=list(range(N_CORES)))
            return _unshard(res.results)
        except Exception as e:  # transient NRT device errors: retry
            last = e
    raise last

